# revision 19
# baseline (speedup 1.0000x reference)
"""Bilateral smoothness loss (BLTSmoothnessLoss) on 8 Trainium2 NeuronCores.

Math (per image):
    pad = reflect_pad(x, 3)                                  # [3, 518, 518]
    for each offset (k, l) in 7x7, (3,3) excluded (d == 0 there):
        d_c    = x_c - pad_c[k:k+512, l:l+512]               # per channel
        S      = sum_c d_c^2
        A      = sum_c |d_c|
        loss  += sum_pixels A * exp(-50*S + ln G[k,l])
    result = total / (8*3*512*512)

Sharding: pure data-parallel over the batch dim (8 images -> 8 cores).
Each core computes a [128,1] f32 partial sum; host reduces.

On-chip layout (per core): 128 partitions, partition p holds padded rows
[4p, 4p+10) of the padded image (4 interior rows + 3-row halo each side),
all 3 channels, with rows padded to 520 cols so the interior starts at an
even (4-byte-aligned) fp16 offset.  A second copy shifted by one column
keeps every shifted view 4-byte aligned (DVE 2x/4x perf modes need it).
"""

import math
import os
import sys

import numpy as np

sys.path.insert(0, "/opt/trn_rl_repo")

import concourse.bass as bass  # noqa: E402
import concourse.mybir as mybir  # noqa: E402
import concourse.tile as tile  # noqa: E402
from concourse.bass_utils import run_bass_kernel_spmd  # noqa: E402

B, C, H, W = 8, 3, 512, 512
FR, K = 3, 7
INV2SR2 = 50.0  # 1 / (2 * 0.1^2)
RPP = 4  # interior rows per partition
HALO = RPP + 2 * FR  # 10 rows incl halo
PW = 520  # padded row width: 1 + 3 | 512 | 3 + 1
PH = H + 2 * FR  # 518 padded rows
CH = HALO * PW  # per-channel free elems per partition
FREE = C * CH  # 15600
NPIX = RPP * W  # 2048 interior pixels per partition per channel
DF16 = mybir.dt.float16
DF32 = mybir.dt.float32
ALU = mybir.AluOpType
ACTF = mybir.ActivationFunctionType


def _gauss():
    m = (K - 1) / 2.0
    y, x = np.ogrid[-m : m + 1, -m : m + 1]
    h = np.exp(-(x * x + y * y) / 2.0)
    h[h < np.finfo(h.dtype).eps * h.max()] = 0
    h /= h.sum()
    return h


GAUSS = _gauss()

# Offsets, center excluded (d == 0 contributes nothing).
OFFSETS = [(k, l) for k in range(K) for l in range(K) if not (k == 3 and l == 3)]


def build_nc(sq_on_act=True, fused_channels=True, split_waits=True):
    from contextlib import ExitStack

    nc = bass.Bass()
    noff = len(OFFSETS)
    xab = nc.declare_dram_parameter("xab", [128, 2 * FREE], DF16, isOutput=False)
    out = nc.declare_dram_parameter("partial", [128, noff], DF32, isOutput=True)

    with ExitStack() as ctx:
        tc = ctx.enter_context(tile.TileContext(nc))
        imgs = ctx.enter_context(tc.tile_pool(name="imgs", bufs=1))
        dpool = ctx.enter_context(tc.tile_pool(name="dp", bufs=2))
        sqpool = ctx.enter_context(tc.tile_pool(name="sqp", bufs=2))
        smalls = ctx.enter_context(tc.tile_pool(name="smalls", bufs=2))
        accp = ctx.enter_context(tc.tile_pool(name="accp", bufs=1))

        AB = imgs.tile([128, 2 * FREE], DF16)
        nc.sync.dma_start(AB[:], xab[:])
        A3 = AB[:, 0:FREE].rearrange("p (c j m) -> p c j m", c=C, j=HALO, m=PW)
        B3 = AB[:, FREE : 2 * FREE].rearrange(
            "p (c j m) -> p c j m", c=C, j=HALO, m=PW
        )
        xv = A3[:, :, FR : FR + RPP, 4 : 4 + W]

        # bias tiles: ln(G) for each unique gaussian value (keyed by radius^2)
        bias_tiles = {}
        for k, l in OFFSETS:
            r2 = (k - 3) ** 2 + (l - 3) ** 2
            if r2 not in bias_tiles:
                t = imgs.tile([128, 1], DF32, tag=f"lng{r2}")
                nc.vector.memset(t[:], float(np.log(GAUSS[k, l])))
                bias_tiles[r2] = t

        acc48 = accp.tile([128, noff], DF32, tag="acc48")
        for idx, (k, l) in enumerate(OFFSETS):
            # shifted view, always starting at an even fp16 offset
            if l % 2 == 1:
                sv = A3[:, :, k : k + RPP, 1 + l : 1 + l + W]
            else:
                sv = B3[:, :, k : k + RPP, l : l + W]

            d = dpool.tile([128, C * NPIX], DF16)
            d3 = d[:].rearrange("p (c j w) -> p c j w", c=C, j=RPP, w=W)
            if fused_channels:
                nc.vector.tensor_tensor(out=d3, in0=xv, in1=sv, op=ALU.subtract)
            else:
                for c in range(C):
                    nc.vector.tensor_tensor(
                        out=d3[:, c], in0=xv[:, c], in1=sv[:, c], op=ALU.subtract
                    )

            sq = sqpool.tile([128, C * NPIX], DF16)
            if sq_on_act:
                nc.scalar.square(sq[:], d[:])
            else:
                nc.vector.tensor_tensor(out=sq[:], in0=d[:], in1=d[:], op=ALU.mult)

            t01 = smalls.tile([128, NPIX], DF16, tag="t01")
            nc.vector.tensor_add(t01[:], sq[:, 0:NPIX], sq[:, NPIX : 2 * NPIX])
            ss = smalls.tile([128, NPIX], DF16, tag="ss")
            nc.vector.tensor_add(ss[:], t01[:], sq[:, 2 * NPIX : 3 * NPIX])

            # |d| for all 3 channels: clear the fp16 sign bit (TS 4x mode)
            av = sqpool.tile([128, C * NPIX], DF16, tag="av")
            nc.vector.tensor_scalar(
                out=av[:].bitcast(mybir.dt.uint16),
                in0=d[:].bitcast(mybir.dt.uint16),
                scalar1=int(0x7FFF),
                scalar2=None,
                op0=ALU.bitwise_and,
            )
            a01 = smalls.tile([128, NPIX], DF16, tag="a01")
            nc.vector.tensor_add(a01[:], av[:, 0:NPIX], av[:, NPIX : 2 * NPIX])
            ad = smalls.tile([128, NPIX], DF16, tag="ad")
            nc.vector.tensor_add(ad[:], a01[:], av[:, 2 * NPIX : 3 * NPIX])

            wt = smalls.tile([128, NPIX], DF16, tag="wt")
            nc.scalar.activation(
                wt[:],
                ss[:],
                ACTF.Exp,
                bias=bias_tiles[(k - 3) ** 2 + (l - 3) ** 2][:],
                scale=-INV2SR2,
            )

            dummy = smalls.tile([128, NPIX], DF16, tag="dummy")
            # dummy = (ad bypass 0) * wt; acc48[:, idx] = sum(dummy)
            nc.vector.scalar_tensor_tensor(
                out=dummy[:],
                in0=ad[:],
                scalar=0.0,
                in1=wt[:],
                op0=ALU.bypass,
                op1=ALU.mult,
                accum_out=acc48[:, idx : idx + 1],
            )

        nc.sync.dma_start(out[:], acc48[:])

    if split_waits:
        _split_excess_waits(nc)
    return nc


def _split_excess_waits(nc):
    """Walrus (this build) allows only one sync-wait per instruction.

    Tile emits up to a few (cross-engine + same-engine). Splitting is
    semantically equivalent: move all but one wait onto single-wait Drain
    instructions inserted just before, on the same engine — engines execute
    their stream in order, so the instruction still starts only after every
    original wait is satisfied.
    """
    for bb in nc.main_func.blocks:
        new_insts = []
        for inst in bb.instructions:
            si = inst.sync_info
            if si is not None and si.on_wait and len(si.on_wait) > 1:
                waits = list(si.on_wait)
                for w in waits[:-1]:
                    d = mybir.InstDrain(
                        name=nc.get_next_instruction_name(),
                        ins=[],
                        outs=[],
                        bass_is_fusable=False,
                    )
                    d.engine = inst.engine
                    d.sync_info = mybir.SyncInfo(on_wait=[w], on_update=[])
                    new_insts.append(d)
                inst.sync_info = mybir.SyncInfo(
                    on_wait=[waits[-1]], on_update=list(si.on_update)
                )
            new_insts.append(inst)
        bb.instructions[:] = new_insts


def prep_core(img):
    """img: [3,512,512] f32 -> (xa, xb) each [128, FREE] fp16."""
    p = np.pad(img, ((0, 0), (FR, FR), (FR, FR)), mode="reflect")  # [3,518,518]
    pw = np.zeros((C, PH, PW), np.float16)
    pw[:, :, 1 : 1 + PH + 2 - 2] = 0  # noop, keep zeros
    pw[:, :, 1:519] = p.astype(np.float16)
    s0, s1, s2 = pw.strides
    av = np.lib.stride_tricks.as_strided(
        pw, shape=(128, C, HALO, PW), strides=(RPP * s1, s0, s1, s2)
    )
    xa = np.ascontiguousarray(av).reshape(128, FREE)
    bw = np.zeros_like(pw)
    bw[:, :, 0:519] = pw[:, :, 1:520]
    bv = np.lib.stride_tricks.as_strided(
        bw, shape=(128, C, HALO, PW), strides=(RPP * s1, s0, s1, s2)
    )
    xb = np.ascontiguousarray(bv).reshape(128, FREE)
    return np.concatenate([xa, xb], axis=1)  # [128, 2*FREE]


_CACHE = {}


def _get_runner():
    """Build the bass program once and return a cached jitted SPMD callable.

    Mirrors concourse.bass2jax.run_bass_via_pjrt, but keeps the jitted
    executable alive so repeated kernel() calls (and timing loops) reuse
    the compiled NEFF instead of re-tracing.
    """
    if "runner" in _CACHE:
        return _CACHE["runner"]

    import jax
    from jax.experimental.shard_map import shard_map
    from jax.sharding import Mesh, PartitionSpec
    from concourse import bass2jax
    import concourse.mybir as mybir_

    bass2jax.install_neuronx_cc_hook()

    nc = build_nc()
    nc.finalize()

    in_names, out_names, out_avals, zero_outs = [], [], [], []
    partition_name = (
        nc.partition_id_tensor.name if nc.partition_id_tensor else None
    )
    for alloc in nc.m.functions[0].allocations:
        if not isinstance(alloc, mybir_.MemoryLocationSet):
            continue
        name = alloc.memorylocations[0].name
        if alloc.kind == "ExternalInput":
            if name != partition_name:
                in_names.append(name)
        elif alloc.kind == "ExternalOutput":
            out_names.append(name)
            shape = tuple(alloc.tensor_shape)
            dtype = mybir_.dt.np(alloc.dtype)
            out_avals.append(jax.core.ShapedArray(shape, dtype))
            zero_outs.append(np.zeros(shape, dtype))
    n_params = len(in_names)
    n_outs = len(out_avals)
    all_names = in_names + out_names
    if partition_name is not None:
        all_names.append(partition_name)
    donate = tuple(range(n_params, n_params + n_outs))

    def _body(*args):
        operands = list(args)
        if partition_name is not None:
            operands.append(bass2jax.partition_id_tensor())
        outs = bass2jax._bass_exec_p.bind(
            *operands,
            out_avals=tuple(out_avals),
            in_names=tuple(all_names),
            out_names=tuple(out_names),
            lowering_input_output_aliases=(),
            sim_require_finite=True,
            sim_require_nnan=True,
            nc=nc,
        )
        return tuple(outs)

    devices = jax.devices()[:B]
    mesh = Mesh(np.asarray(devices), ("core",))
    in_specs = (PartitionSpec("core"),) * (n_params + n_outs)
    out_specs = (PartitionSpec("core"),) * n_outs
    sharded = jax.jit(
        shard_map(
            _body, mesh=mesh, in_specs=in_specs, out_specs=out_specs,
            check_rep=False,
        ),
        donate_argnums=donate,
        keep_unused=True,
    )

    def run(in_maps, timing_reps=0):
        concat_in = [
            np.concatenate([np.asarray(m[name]) for m in in_maps], axis=0)
            for name in in_names
        ]
        concat_zeros = [
            np.zeros((B * z.shape[0], *z.shape[1:]), z.dtype) for z in zero_outs
        ]
        times = []
        if timing_reps > 0:
            # stage inputs on device once so repeat calls time dispatch+exec
            import time as _time

            sharding = jax.sharding.NamedSharding(mesh, PartitionSpec("core"))
            dev_in = [jax.device_put(a, sharding) for a in concat_in]
            for a in dev_in:
                a.block_until_ready()
            for _ in range(timing_reps):
                dz = [jax.device_put(z, sharding) for z in concat_zeros]
                for z in dz:
                    z.block_until_ready()
                t0 = _time.time()
                outs = sharded(*dev_in, *dz)
                for o in outs:
                    o.block_until_ready()
                times.append(_time.time() - t0)
        out_arrs = sharded(*concat_in, *concat_zeros)
        out_arrs = [np.asarray(o) for o in out_arrs]
        results = [
            {
                name: out_arrs[i].reshape(B, *out_avals[i].shape)[c]
                for i, name in enumerate(out_names)
            }
            for c in range(B)
        ]
        return results, times

    _CACHE["runner"] = run
    return run


def run_on_cores(x, repeats=0):
    """x: [8,3,512,512] f32 numpy. Returns (loss, exec_times_s list)."""
    in_maps = [{"xab": prep_core(x[b])} for b in range(B)]
    run = _get_runner()
    results, times = run(in_maps, timing_reps=repeats)
    total = 0.0
    for r in results:
        total += float(r["partial"].astype(np.float64).sum())
    loss = np.float32(total / (B * C * H * W))
    return loss, times


def kernel(input):
    x = np.asarray(input, dtype=np.float32)
    assert x.shape == (B, C, H, W), x.shape
    loss, _ = run_on_cores(x)
    return loss


# revision 26
# speedup vs baseline: 150.7688x; 150.7688x over previous
"""Bilateral smoothness loss (BLTSmoothnessLoss) on 8 Trainium2 NeuronCores.

Math (per image):
    pad = reflect_pad(x, 3)                                  # [3, 518, 518]
    for each offset (k, l) in 7x7, (3,3) excluded (d == 0 there):
        d_c    = x_c - pad_c[k:k+512, l:l+512]               # per channel
        S      = sum_c d_c^2
        A      = sum_c |d_c|
        loss  += sum_pixels A * exp(-50*S + ln G[k,l])
    result = total / (8*3*512*512)

Sharding: pure data-parallel over the batch dim (8 images -> 8 cores).
Each core computes a [128,1] f32 partial sum; host reduces.

On-chip layout (per core): 128 partitions, partition p holds padded rows
[4p, 4p+10) of the padded image (4 interior rows + 3-row halo each side),
all 3 channels, with rows padded to 520 cols so the interior starts at an
even (4-byte-aligned) fp16 offset.  A second copy shifted by one column
keeps every shifted view 4-byte aligned (DVE 2x/4x perf modes need it).
"""

import math
import os
import sys

import numpy as np

sys.path.insert(0, "/opt/trn_rl_repo")

import concourse.bass as bass  # noqa: E402
import concourse.mybir as mybir  # noqa: E402
import concourse.tile as tile  # noqa: E402
from concourse.bass_utils import run_bass_kernel_spmd  # noqa: E402

B, C, H, W = 8, 3, 512, 512
FR, K = 3, 7
INV2SR2 = 50.0  # 1 / (2 * 0.1^2)
RPP = 4  # interior rows per partition
HALO = RPP + 2 * FR  # 10 rows incl halo
PW = 520  # padded row width: 1 + 3 | 512 | 3 + 1
PH = H + 2 * FR  # 518 padded rows
CH = HALO * PW  # per-channel free elems per partition
FREE = C * CH  # 15600
NPIX = RPP * W  # 2048 interior pixels per partition per channel
DF16 = mybir.dt.float16
DF32 = mybir.dt.float32
ALU = mybir.AluOpType
ACTF = mybir.ActivationFunctionType


def _gauss():
    m = (K - 1) / 2.0
    y, x = np.ogrid[-m : m + 1, -m : m + 1]
    h = np.exp(-(x * x + y * y) / 2.0)
    h[h < np.finfo(h.dtype).eps * h.max()] = 0
    h /= h.sum()
    return h


GAUSS = _gauss()

# Offsets, center excluded (d == 0 contributes nothing).
OFFSETS = [(k, l) for k in range(K) for l in range(K) if not (k == 3 and l == 3)]


def build_nc(sq_on_act=True, fused_channels=True, split_waits=True, repeat=1):
    from contextlib import ExitStack

    nc = bass.Bass()
    noff = len(OFFSETS)
    xab = nc.declare_dram_parameter("xab", [128, 2 * FREE], DF16, isOutput=False)
    out = nc.declare_dram_parameter("partial", [128, noff], DF32, isOutput=True)

    with ExitStack() as ctx:
        tc = ctx.enter_context(tile.TileContext(nc))
        imgs = ctx.enter_context(tc.tile_pool(name="imgs", bufs=1))
        dpool = ctx.enter_context(tc.tile_pool(name="dp", bufs=2))
        sqpool = ctx.enter_context(tc.tile_pool(name="sqp", bufs=2))
        smalls = ctx.enter_context(tc.tile_pool(name="smalls", bufs=2))
        accp = ctx.enter_context(tc.tile_pool(name="accp", bufs=1))

        AB = imgs.tile([128, 2 * FREE], DF16)
        nc.sync.dma_start(AB[:], xab[:])
        A3 = AB[:, 0:FREE].rearrange("p (c j m) -> p c j m", c=C, j=HALO, m=PW)
        B3 = AB[:, FREE : 2 * FREE].rearrange(
            "p (c j m) -> p c j m", c=C, j=HALO, m=PW
        )
        xv = A3[:, :, FR : FR + RPP, 4 : 4 + W]

        # bias tiles: ln(G) for each unique gaussian value (keyed by radius^2)
        bias_tiles = {}
        for k, l in OFFSETS:
            r2 = (k - 3) ** 2 + (l - 3) ** 2
            if r2 not in bias_tiles:
                t = imgs.tile([128, 1], DF32, tag=f"lng{r2}")
                nc.vector.memset(t[:], float(np.log(GAUSS[k, l])))
                bias_tiles[r2] = t

        acc48 = accp.tile([128, noff], DF32, tag="acc48")

        def emit_offset(idx, k, l):
            # shifted view, always starting at an even fp16 offset
            if l % 2 == 1:
                sv = A3[:, :, k : k + RPP, 1 + l : 1 + l + W]
            else:
                sv = B3[:, :, k : k + RPP, l : l + W]

            d = dpool.tile([128, C * NPIX], DF16)
            d3 = d[:].rearrange("p (c j w) -> p c j w", c=C, j=RPP, w=W)
            if fused_channels:
                nc.vector.tensor_tensor(out=d3, in0=xv, in1=sv, op=ALU.subtract)
            else:
                for c in range(C):
                    nc.vector.tensor_tensor(
                        out=d3[:, c], in0=xv[:, c], in1=sv[:, c], op=ALU.subtract
                    )

            sq = sqpool.tile([128, C * NPIX], DF16)
            if sq_on_act:
                nc.scalar.square(sq[:], d[:])
            else:
                nc.vector.tensor_tensor(out=sq[:], in0=d[:], in1=d[:], op=ALU.mult)

            t01 = smalls.tile([128, NPIX], DF16, tag="t01")
            nc.vector.tensor_add(t01[:], sq[:, 0:NPIX], sq[:, NPIX : 2 * NPIX])
            ss = smalls.tile([128, NPIX], DF16, tag="ss")
            nc.vector.tensor_add(ss[:], t01[:], sq[:, 2 * NPIX : 3 * NPIX])

            # |d| for all 3 channels: clear the fp16 sign bit (TS 4x mode)
            av = sqpool.tile([128, C * NPIX], DF16, tag="av")
            nc.vector.tensor_scalar(
                out=av[:].bitcast(mybir.dt.uint16),
                in0=d[:].bitcast(mybir.dt.uint16),
                scalar1=int(0x7FFF),
                scalar2=None,
                op0=ALU.bitwise_and,
            )
            a01 = smalls.tile([128, NPIX], DF16, tag="a01")
            nc.vector.tensor_add(a01[:], av[:, 0:NPIX], av[:, NPIX : 2 * NPIX])
            ad = smalls.tile([128, NPIX], DF16, tag="ad")
            nc.vector.tensor_add(ad[:], a01[:], av[:, 2 * NPIX : 3 * NPIX])

            wt = smalls.tile([128, NPIX], DF16, tag="wt")
            nc.scalar.activation(
                wt[:],
                ss[:],
                ACTF.Exp,
                bias=bias_tiles[(k - 3) ** 2 + (l - 3) ** 2][:],
                scale=-INV2SR2,
            )

            dummy = smalls.tile([128, NPIX], DF16, tag="dummy")
            # dummy = (ad bypass 0) * wt; acc48[:, idx] = sum(dummy)
            nc.vector.scalar_tensor_tensor(
                out=dummy[:],
                in0=ad[:],
                scalar=0.0,
                in1=wt[:],
                op0=ALU.bypass,
                op1=ALU.mult,
                accum_out=acc48[:, idx : idx + 1],
            )

        def emit_all():
            for idx, (k, l) in enumerate(OFFSETS):
                emit_offset(idx, k, l)

        if repeat > 1:
            with tc.For_i(0, repeat, 1):
                emit_all()
        else:
            emit_all()

        nc.sync.dma_start(out[:], acc48[:])

    if split_waits:
        _split_excess_waits(nc)
    return nc


def _split_excess_waits(nc):
    """Walrus (this build) allows only one sync-wait per instruction.

    Tile emits up to a few (cross-engine + same-engine). Splitting is
    semantically equivalent: move all but one wait onto single-wait Drain
    instructions inserted just before, on the same engine — engines execute
    their stream in order, so the instruction still starts only after every
    original wait is satisfied.
    """
    for bb in nc.main_func.blocks:
        new_insts = []
        for inst in bb.instructions:
            si = inst.sync_info
            if si is not None and si.on_wait and len(si.on_wait) > 1:
                waits = list(si.on_wait)
                for w in waits[:-1]:
                    d = mybir.InstDrain(
                        name=nc.get_next_instruction_name(),
                        ins=[],
                        outs=[],
                        bass_is_fusable=False,
                    )
                    d.engine = inst.engine
                    d.sync_info = mybir.SyncInfo(on_wait=[w], on_update=[])
                    new_insts.append(d)
                inst.sync_info = mybir.SyncInfo(
                    on_wait=[waits[-1]], on_update=list(si.on_update)
                )
            new_insts.append(inst)
        bb.instructions[:] = new_insts


def prep_core(img):
    """img: [3,512,512] f32 -> (xa, xb) each [128, FREE] fp16."""
    p = np.pad(img, ((0, 0), (FR, FR), (FR, FR)), mode="reflect")  # [3,518,518]
    pw = np.zeros((C, PH, PW), np.float16)
    pw[:, :, 1 : 1 + PH + 2 - 2] = 0  # noop, keep zeros
    pw[:, :, 1:519] = p.astype(np.float16)
    s0, s1, s2 = pw.strides
    av = np.lib.stride_tricks.as_strided(
        pw, shape=(128, C, HALO, PW), strides=(RPP * s1, s0, s1, s2)
    )
    xa = np.ascontiguousarray(av).reshape(128, FREE)
    bw = np.zeros_like(pw)
    bw[:, :, 0:519] = pw[:, :, 1:520]
    bv = np.lib.stride_tricks.as_strided(
        bw, shape=(128, C, HALO, PW), strides=(RPP * s1, s0, s1, s2)
    )
    xb = np.ascontiguousarray(bv).reshape(128, FREE)
    return np.concatenate([xa, xb], axis=1)  # [128, 2*FREE]


_CACHE = {}


def _get_runner(repeat=1):
    """Build the bass program once and return a cached jitted SPMD callable.

    Mirrors concourse.bass2jax.run_bass_via_pjrt, but keeps the jitted
    executable alive so repeated kernel() calls (and timing loops) reuse
    the compiled NEFF instead of re-tracing.
    """
    key = f"runner{repeat}"
    if key in _CACHE:
        return _CACHE[key]

    import jax
    from jax.experimental.shard_map import shard_map
    from jax.sharding import Mesh, PartitionSpec
    from concourse import bass2jax
    import concourse.mybir as mybir_

    bass2jax.install_neuronx_cc_hook()

    nc = build_nc(repeat=repeat)
    nc.finalize()

    in_names, out_names, out_avals, zero_outs = [], [], [], []
    partition_name = (
        nc.partition_id_tensor.name if nc.partition_id_tensor else None
    )
    for alloc in nc.m.functions[0].allocations:
        if not isinstance(alloc, mybir_.MemoryLocationSet):
            continue
        name = alloc.memorylocations[0].name
        if alloc.kind == "ExternalInput":
            if name != partition_name:
                in_names.append(name)
        elif alloc.kind == "ExternalOutput":
            out_names.append(name)
            shape = tuple(alloc.tensor_shape)
            dtype = mybir_.dt.np(alloc.dtype)
            out_avals.append(jax.core.ShapedArray(shape, dtype))
            zero_outs.append(np.zeros(shape, dtype))
    n_params = len(in_names)
    n_outs = len(out_avals)
    all_names = in_names + out_names
    if partition_name is not None:
        all_names.append(partition_name)
    donate = tuple(range(n_params, n_params + n_outs))

    def _body(*args):
        operands = list(args)
        if partition_name is not None:
            operands.append(bass2jax.partition_id_tensor())
        outs = bass2jax._bass_exec_p.bind(
            *operands,
            out_avals=tuple(out_avals),
            in_names=tuple(all_names),
            out_names=tuple(out_names),
            lowering_input_output_aliases=(),
            sim_require_finite=True,
            sim_require_nnan=True,
            nc=nc,
        )
        return tuple(outs)

    devices = jax.devices()[:B]
    mesh = Mesh(np.asarray(devices), ("core",))
    in_specs = (PartitionSpec("core"),) * (n_params + n_outs)
    out_specs = (PartitionSpec("core"),) * n_outs
    sharded = jax.jit(
        shard_map(
            _body, mesh=mesh, in_specs=in_specs, out_specs=out_specs,
            check_rep=False,
        ),
        donate_argnums=donate,
        keep_unused=True,
    )

    def run(in_maps, timing_reps=0):
        concat_in = [
            np.concatenate([np.asarray(m[name]) for m in in_maps], axis=0)
            for name in in_names
        ]
        concat_zeros = [
            np.zeros((B * z.shape[0], *z.shape[1:]), z.dtype) for z in zero_outs
        ]
        times = []
        if timing_reps > 0:
            # stage inputs on device once so repeat calls time dispatch+exec
            import time as _time

            sharding = jax.sharding.NamedSharding(mesh, PartitionSpec("core"))
            dev_in = [jax.device_put(a, sharding) for a in concat_in]
            for a in dev_in:
                a.block_until_ready()
            for _ in range(timing_reps):
                dz = [jax.device_put(z, sharding) for z in concat_zeros]
                for z in dz:
                    z.block_until_ready()
                t0 = _time.time()
                outs = sharded(*dev_in, *dz)
                for o in outs:
                    o.block_until_ready()
                times.append(_time.time() - t0)
        out_arrs = sharded(*concat_in, *concat_zeros)
        out_arrs = [np.asarray(o) for o in out_arrs]
        results = [
            {
                name: out_arrs[i].reshape(B, *out_avals[i].shape)[c]
                for i, name in enumerate(out_names)
            }
            for c in range(B)
        ]
        return results, times

    _CACHE[key] = run
    return run


def measure_exec_s(x, n=33, reps=6):
    """Time the kernel body on hardware via an on-device repeat loop.

    Builds two NEFFs: the normal one (repeat=1) and one whose offset sweep
    runs `n` times in a For_i loop.  (t_n - t_1) / (n - 1) cancels the
    dispatch/tunnel overhead, which dwarfs the kernel itself.  Calls are
    timed with device-staged inputs (timing_reps path).
    """
    in_maps = [{"xab": prep_core(x[b])} for b in range(B)]
    results = {}
    for cnt in (1, n):
        run = _get_runner(repeat=cnt)
        _, times = run(in_maps, timing_reps=reps)
        results[cnt] = min(times)
    per_iter = (results[n] - results[1]) / (n - 1)
    return per_iter, results


def run_on_cores(x, repeats=0):
    """x: [8,3,512,512] f32 numpy. Returns (loss, exec_times_s list)."""
    in_maps = [{"xab": prep_core(x[b])} for b in range(B)]
    run = _get_runner()
    results, times = run(in_maps, timing_reps=repeats)
    total = 0.0
    for r in results:
        total += float(r["partial"].astype(np.float64).sum())
    loss = np.float32(total / (B * C * H * W))
    return loss, times


def kernel(input):
    x = np.asarray(input, dtype=np.float32)
    assert x.shape == (B, C, H, W), x.shape
    loss, _ = run_on_cores(x)
    return loss


# revision 52
# speedup vs baseline: 543.9784x; 3.6080x over previous
"""Bilateral smoothness loss (BLTSmoothnessLoss) on 8 Trainium2 NeuronCores.

Math (per image):
    pad = reflect_pad(x, 3)                                  # [3, 518, 518]
    for each offset (k, l) in 7x7, (3,3) excluded (d == 0 there):
        d_c    = x_c - pad_c[k:k+512, l:l+512]               # per channel
        S      = sum_c d_c^2
        A      = sum_c |d_c|
        loss  += sum_pixels A * exp(-50*S + ln G[k,l])
    result = total / (8*3*512*512)

Sharding: pure data-parallel over the batch dim (8 images -> 8 cores).
Each core computes a [128,1] f32 partial sum; host reduces.

On-chip layout (per core): 128 partitions, partition p holds padded rows
[4p, 4p+10) of the padded image (4 interior rows + 3-row halo each side),
all 3 channels, with rows padded to 520 cols so the interior starts at an
even (4-byte-aligned) fp16 offset.  A second copy shifted by one column
keeps every shifted view 4-byte aligned (DVE 2x/4x perf modes need it).
"""

import math
import os
import sys

import numpy as np

sys.path.insert(0, "/opt/trn_rl_repo")

import concourse.bass as bass  # noqa: E402
import concourse.mybir as mybir  # noqa: E402
import concourse.tile as tile  # noqa: E402
from concourse.bass_utils import run_bass_kernel_spmd  # noqa: E402

B, C, H, W = 8, 3, 512, 512
FR, K = 3, 7
INV2SR2 = 50.0  # 1 / (2 * 0.1^2)
RPP = 4  # interior rows per partition
HALO = RPP + 2 * FR  # 10 rows incl halo
PW = 520  # padded row width: 1 + 3 | 512 | 3 + 1
PH = H + 2 * FR  # 518 padded rows
CH = HALO * PW  # per-channel free elems per partition
FREE = C * CH  # 15600
NPIX = RPP * W  # 2048 interior pixels per partition per channel
DF16 = mybir.dt.float16
DF32 = mybir.dt.float32
ALU = mybir.AluOpType
ACTF = mybir.ActivationFunctionType


def _gauss():
    m = (K - 1) / 2.0
    y, x = np.ogrid[-m : m + 1, -m : m + 1]
    h = np.exp(-(x * x + y * y) / 2.0)
    h[h < np.finfo(h.dtype).eps * h.max()] = 0
    h /= h.sum()
    return h


GAUSS = _gauss()

# Offsets, center excluded (d == 0 contributes nothing).
def make_offsets(drop_r2=99):
    """All 7x7 taps except the center; taps with radius^2 >= drop_r2 dropped.

    The 4 corner taps (r2=18) carry G=1.96e-5 each — 9.3e-5 of the loss —
    far below fp16 noise, so dropping them is free accuracy-wise.
    """
    out = []
    for k in range(K):
        for l in range(K):
            if k == 3 and l == 3:
                continue
            if (k - 3) ** 2 + (l - 3) ** 2 >= drop_r2:
                continue
            out.append((k, l))
    return out


OFFSETS = make_offsets()

# --- symmetric (pair) formulation ---------------------------------------
# pair_total(o) = 2*full(o) - sum_{B_o} f_o + sum_{B_-o} f_-o  (validated
# exactly in sym_check.py).  B_sigma = pixels whose +sigma partner is
# reflected.  Corrections are thin strips; row strips are computed in a
# column-partitioned "band" workspace so their FD stays tiny per partition.
BAND_J = 18  # 9 top padded rows (0..8) + 9 bottom (509..517)
# per-partition cols: m 0..9 = local cols (pw col 4q+1+m); m 10..19 = the
# right-edge cols pw[509..518] replicated everywhere, so right-edge corner
# regions can be addressed from partition 0 (APs must start at partition 0).
BAND_M = 20
BAND_FREE = C * BAND_J * BAND_M


def make_pairs(drop_r2=99):
    out = []
    for a in range(0, 4):
        for b in range(-3, 4):
            if (a, b) <= (0, 0):
                continue
            if a * a + b * b >= drop_r2:
                continue
            out.append((a, b))
    return out


def _corr_plan(drop_r2):
    """Slot layout for the correction workspace.

    Returns (slots_total, sigma_list) where each sigma entry is
    (a, b, sign, G, regions) and regions is a list of
    (kind, slot_off, width) with kind in {"rs", "cs", "corner"}.
    rs+cs slots are adjacent so one accumulation covers both.
    """
    off = 0
    sigmas = []
    for a, b in make_pairs(drop_r2):
        for sa, sb, sign in ((a, b, -1.0), (-a, -b, 1.0)):
            G = float(GAUSS[sa + 3, sb + 3])
            regions = []
            strip_w = 0
            if sa != 0:
                regions.append(("rs", off + strip_w, abs(sa) * RPP))
                strip_w += abs(sa) * RPP
            if sb != 0:
                regions.append(("cs", off + strip_w, abs(sb) * RPP))
                strip_w += abs(sb) * RPP
            corner_w = 0
            if sa != 0 and sb != 0:
                regions.append(("corner", off + strip_w, abs(sa) * abs(sb)))
                corner_w = abs(sa) * abs(sb)
            sigmas.append((sa, sb, sign, G, regions, off, strip_w, corner_w))
            off += strip_w + corner_w
    return off, sigmas


def build_nc(sq_on_act=True, fused_channels=False, split_waits=True, repeat=1,
             drop_r2=99, nbufs=2, gpsimd_adds=False, sym=False,
             act_accum=False, split_dma=False, stt_bcast=False,
             pe_sums=0, sq_bufs=2):
    from contextlib import ExitStack

    if sym:
        mains = [(a + 3, b + 3) for a, b in make_pairs(drop_r2)]
        SLOTS, sigmas = _corr_plan(drop_r2)
        n_acc = len(mains) + sum(
            1 + (1 if s[7] else 0) for s in sigmas
        )
    else:
        mains = make_offsets(drop_r2)
        SLOTS, sigmas = 0, []
        n_acc = len(mains)

    if split_dma:
        # A-only offsets (odd l -> shifted view reads copy A) first, so
        # their compute overlaps the B-copy DMA
        mains = sorted(mains, key=lambda kl: 0 if kl[1] % 2 == 1 else 1)
    offs = mains
    nc = bass.Bass()
    noff = n_acc
    xab = nc.declare_dram_parameter("xab", [128, 2 * FREE], DF16, isOutput=False)
    if sym:
        bands = nc.declare_dram_parameter(
            "bands", [128, BAND_FREE], DF16, isOutput=False
        )
    if pe_sums:
        identp = nc.declare_dram_parameter("ident", [128, 128], DF16, isOutput=False)
    out = nc.declare_dram_parameter("partial", [128, noff], DF32, isOutput=True)

    with ExitStack() as ctx:
        tc = ctx.enter_context(tile.TileContext(nc))
        imgs = ctx.enter_context(tc.tile_pool(name="imgs", bufs=1))
        dpool = ctx.enter_context(tc.tile_pool(name="dp", bufs=nbufs))
        sqpool = ctx.enter_context(tc.tile_pool(name="sqp", bufs=sq_bufs))
        smalls = ctx.enter_context(tc.tile_pool(name="smalls", bufs=nbufs))
        accp = ctx.enter_context(tc.tile_pool(name="accp", bufs=1))

        AB = imgs.tile([128, 2 * FREE], DF16)
        if split_dma:
            nc.sync.dma_start(AB[:, 0:FREE], xab[:, 0:FREE])
            nc.sync.dma_start(AB[:, FREE : 2 * FREE], xab[:, FREE : 2 * FREE])
        else:
            nc.sync.dma_start(AB[:], xab[:])
        A3 = AB[:, 0:FREE].rearrange("p (c j m) -> p c j m", c=C, j=HALO, m=PW)
        B3 = AB[:, FREE : 2 * FREE].rearrange(
            "p (c j m) -> p c j m", c=C, j=HALO, m=PW
        )
        xv = A3[:, :, FR : FR + RPP, 4 : 4 + W]
        if sym:
            BandT = imgs.tile([128, BAND_FREE], DF16, tag="band")
            nc.sync.dma_start(BandT[:], bands[:])
            Bnd3 = BandT[:].rearrange(
                "p (c j m) -> p c j m", c=C, j=BAND_J, m=BAND_M
            )
            wsp = ctx.enter_context(tc.tile_pool(name="wsp", bufs=1))

        # bias tiles: ln(G) (ln(2G) for sym mains) keyed by radius^2
        gmul = 2.0 if sym else 1.0
        bias_tiles = {}
        for k, l in offs:
            r2 = (k - 3) ** 2 + (l - 3) ** 2
            if r2 not in bias_tiles:
                t = imgs.tile([128, 1], DF32, tag=f"lng{r2}")
                nc.vector.memset(t[:], float(np.log(gmul * GAUSS[k, l])))
                bias_tiles[r2] = t

        acc48 = accp.tile([128, noff], DF32, tag="acc48")
        if sym:
            nc.vector.memset(acc48[:], 0.0)
        if pe_sums:
            ident = imgs.tile([128, 128], DF16, tag="ident")
            nc.sync.dma_start(ident[:], identp[:])
            psums = ctx.enter_context(
                tc.tile_pool(name="psum", bufs=1, space="PSUM")
            )

        def emit_offset(idx, k, l):
            # shifted view, always starting at an even fp16 offset
            if l % 2 == 1:
                sv = A3[:, :, k : k + RPP, 1 + l : 1 + l + W]
            else:
                sv = B3[:, :, k : k + RPP, l : l + W]

            d = dpool.tile([128, C * NPIX], DF16)
            d3 = d[:].rearrange("p (c j w) -> p c j w", c=C, j=RPP, w=W)
            if fused_channels:
                nc.vector.tensor_tensor(out=d3, in0=xv, in1=sv, op=ALU.subtract)
            else:
                for c in range(C):
                    nc.vector.tensor_tensor(
                        out=d3[:, c], in0=xv[:, c], in1=sv[:, c], op=ALU.subtract
                    )

            sq = sqpool.tile([128, C * NPIX], DF16, tag="sq")
            if sq_on_act:
                nc.scalar.square(sq[:], d[:])
            else:
                nc.vector.tensor_tensor(out=sq[:], in0=d[:], in1=d[:], op=ALU.mult)

            if pe_sums:
                ss = psums.tile([128, NPIX], DF32, tag="ssp")
                for j in range(NPIX // 512):
                    for c in range(C):
                        nc.tensor.matmul(
                            out=ss[:, j * 512 : (j + 1) * 512],
                            lhsT=ident[:],
                            rhs=sq[:, c * NPIX + j * 512 : c * NPIX + (j + 1) * 512],
                            start=(c == 0),
                            stop=(c == C - 1),
                        )
            else:
                t01 = smalls.tile([128, NPIX], DF16, tag="t01")
                nc.vector.tensor_add(t01[:], sq[:, 0:NPIX], sq[:, NPIX : 2 * NPIX])
                ss = smalls.tile([128, NPIX], DF16, tag="ss")
                nc.vector.tensor_add(ss[:], t01[:], sq[:, 2 * NPIX : 3 * NPIX])

            # |d| for all 3 channels: clear the fp16 sign bit (TS 4x mode)
            av = sqpool.tile([128, C * NPIX], DF16, tag="av")
            nc.vector.tensor_scalar(
                out=av[:].bitcast(mybir.dt.uint16),
                in0=d[:].bitcast(mybir.dt.uint16),
                scalar1=int(0x7FFF),
                scalar2=None,
                op0=ALU.bitwise_and,
            )
            if pe_sums >= 2:
                ad = psums.tile([128, NPIX], DF32, tag="adp")
                for j in range(NPIX // 512):
                    for c in range(C):
                        nc.tensor.matmul(
                            out=ad[:, j * 512 : (j + 1) * 512],
                            lhsT=ident[:],
                            rhs=av[:, c * NPIX + j * 512 : c * NPIX + (j + 1) * 512],
                            start=(c == 0),
                            stop=(c == C - 1),
                        )
            el_null = None
            if pe_sums >= 2:
                pass
            elif gpsimd_adds:
                a01 = smalls.tile([128, NPIX], DF16, tag="a01")
                ad = smalls.tile([128, NPIX], DF16, tag="ad")
                nc.gpsimd.tensor_tensor(
                    out=a01[:], in0=av[:, 0:NPIX], in1=av[:, NPIX : 2 * NPIX],
                    op=ALU.add)
                nc.gpsimd.tensor_tensor(
                    out=ad[:], in0=a01[:], in1=av[:, 2 * NPIX : 3 * NPIX],
                    op=ALU.add)
            else:
                a01 = smalls.tile([128, NPIX], DF16, tag="a01")
                ad = smalls.tile([128, NPIX], DF16, tag="ad")
                nc.vector.tensor_add(a01[:], av[:, 0:NPIX], av[:, NPIX : 2 * NPIX])
                nc.vector.tensor_add(ad[:], a01[:], av[:, 2 * NPIX : 3 * NPIX])

            wt = smalls.tile([128, NPIX], DF16, tag="wt")
            nc.scalar.activation(
                wt[:],
                ss[:],
                ACTF.Exp,
                bias=bias_tiles[(k - 3) ** 2 + (l - 3) ** 2][:],
                scale=-INV2SR2,
            )

            if act_accum:
                dummy = smalls.tile([128, NPIX], DF16, tag="dummy")
                nc.vector.tensor_tensor(
                    out=dummy[:], in0=ad[:], in1=wt[:], op=ALU.mult
                )
                nc.scalar.activation(
                    dummy[:], dummy[:], ACTF.Copy,
                    accum_out=acc48[:, idx : idx + 1],
                )
            elif stt_bcast:
                dummy = smalls.tile([128, 1], DF16, tag="dummy1")
                nc.vector.scalar_tensor_tensor(
                    out=dummy[:].broadcast_to((128, NPIX)),
                    in0=ad[:],
                    scalar=0.0,
                    in1=wt[:],
                    op0=ALU.bypass,
                    op1=ALU.mult,
                    accum_out=acc48[:, idx : idx + 1],
                )
            else:
                dummy = smalls.tile([128, NPIX], DF16, tag="dummy")
                # dummy = (ad bypass 0) * wt; acc48[:, idx] = sum(dummy)
                nc.vector.scalar_tensor_tensor(
                    out=dummy[:],
                    in0=ad[:],
                    scalar=0.0,
                    in1=wt[:],
                    op0=ALU.bypass,
                    op1=ALU.mult,
                    accum_out=acc48[:, idx : idx + 1],
                )

        def emit_corrections():
            U16 = mybir.dt.uint16
            dW = wsp.tile([128, C * SLOTS], DF16, tag="dW")
            nc.vector.memset(dW[:], 0.0)
            dW3 = dW[:].rearrange("p (c s) -> p c s", c=C, s=SLOTS)

            def corner_part(sb):
                # left corner: x cols 0..|sb|-1 -> m = c'+3 (partition 0);
                # right corner: x cols 512-sb..511 -> replicated slots,
                # m = c'-495 (also partition 0)
                return (0, 3) if sb < 0 else (0, 17 - sb)

            for sa, sb, sign, G, regions, off0, strip_w, corner_w in sigmas:
                for kind, off, wdt in regions:
                    if kind == "rs":
                        na = abs(sa)
                        j0 = 3 if sa < 0 else 15 - sa
                        xw = Bnd3[:, :, j0 : j0 + na, 3:7]
                        sw = Bnd3[:, :, j0 + sa : j0 + sa + na, 3 + sb : 7 + sb]
                        ow = dW3[:, :, off : off + wdt].rearrange(
                            "p c (r w) -> p c r w", r=na, w=RPP
                        )
                    elif kind == "cs":
                        nb = abs(sb)
                        c0 = 0 if sb < 0 else W - nb
                        k_ = sa + 3
                        xw = A3[:, :, 3:7, 4 + c0 : 4 + c0 + nb]
                        sw = A3[:, :, k_ : k_ + RPP, 4 + sb + c0 : 4 + sb + c0 + nb]
                        ow = dW3[:, :, off : off + wdt].rearrange(
                            "p c (r w) -> p c r w", r=RPP, w=nb
                        )
                    else:  # corner
                        na, nb = abs(sa), abs(sb)
                        j0 = 3 if sa < 0 else 15 - sa
                        q_, m0 = corner_part(sb)
                        xw = Bnd3[q_ : q_ + 1, :, j0 : j0 + na, m0 : m0 + nb]
                        sw = Bnd3[
                            q_ : q_ + 1, :,
                            j0 + sa : j0 + sa + na,
                            m0 + sb : m0 + sb + nb,
                        ]
                        ow = dW3[q_ : q_ + 1, :, off : off + wdt].rearrange(
                            "p c (r w) -> p c r w", r=na, w=nb
                        )
                    nc.vector.tensor_tensor(out=ow, in0=xw, in1=sw, op=ALU.subtract)

            sqW = wsp.tile([128, C * SLOTS], DF16, tag="sqW")
            nc.scalar.square(sqW[:], dW[:])
            t01W = wsp.tile([128, SLOTS], DF16, tag="t01W")
            nc.vector.tensor_add(t01W[:], sqW[:, 0:SLOTS], sqW[:, SLOTS : 2 * SLOTS])
            ssW = wsp.tile([128, SLOTS], DF16, tag="ssW")
            nc.vector.tensor_add(ssW[:], t01W[:], sqW[:, 2 * SLOTS : 3 * SLOTS])
            avW = wsp.tile([128, C * SLOTS], DF16, tag="avW")
            nc.vector.tensor_scalar(
                out=avW[:].bitcast(U16),
                in0=dW[:].bitcast(U16),
                scalar1=int(0x7FFF),
                scalar2=None,
                op0=ALU.bitwise_and,
            )
            a01W = wsp.tile([128, SLOTS], DF16, tag="a01W")
            nc.vector.tensor_add(a01W[:], avW[:, 0:SLOTS], avW[:, SLOTS : 2 * SLOTS])
            adW = wsp.tile([128, SLOTS], DF16, tag="adW")
            nc.vector.tensor_add(adW[:], a01W[:], avW[:, 2 * SLOTS : 3 * SLOTS])
            wtW = wsp.tile([128, SLOTS], DF16, tag="wtW")
            nc.scalar.activation(wtW[:], ssW[:], ACTF.Exp, bias=0.0, scale=-INV2SR2)

            dumW = wsp.tile([128, SLOTS], DF16, tag="dumW")
            col = len(mains)
            for sa, sb, sign, G, regions, off0, strip_w, corner_w in sigmas:
                nc.vector.scalar_tensor_tensor(
                    out=dumW[:, off0 : off0 + strip_w],
                    in0=adW[:, off0 : off0 + strip_w],
                    scalar=sign * G,
                    in1=wtW[:, off0 : off0 + strip_w],
                    op0=ALU.mult,
                    op1=ALU.mult,
                    accum_out=acc48[:, col : col + 1],
                )
                col += 1
                if corner_w:
                    q_, _ = corner_part(sb)
                    co = off0 + strip_w
                    nc.vector.scalar_tensor_tensor(
                        out=dumW[q_ : q_ + 1, co : co + corner_w],
                        in0=adW[q_ : q_ + 1, co : co + corner_w],
                        scalar=-sign * G,
                        in1=wtW[q_ : q_ + 1, co : co + corner_w],
                        op0=ALU.mult,
                        op1=ALU.mult,
                        accum_out=acc48[q_ : q_ + 1, col : col + 1],
                    )
                    col += 1

        def emit_all():
            for idx, (k, l) in enumerate(offs):
                emit_offset(idx, k, l)
            if sym:
                emit_corrections()

        if repeat > 1:
            with tc.For_i(0, repeat, 1):
                emit_all()
        else:
            emit_all()

        nc.sync.dma_start(out[:], acc48[:])

    if split_waits:
        _split_excess_waits(nc)
    return nc


def _split_excess_waits(nc):
    """Walrus (this build) allows only one sync-wait per instruction.

    Tile emits up to a few (cross-engine + same-engine). Splitting is
    semantically equivalent: move all but one wait onto single-wait Drain
    instructions inserted just before, on the same engine — engines execute
    their stream in order, so the instruction still starts only after every
    original wait is satisfied.
    """
    for bb in nc.main_func.blocks:
        new_insts = []
        for inst in bb.instructions:
            si = inst.sync_info
            if si is not None and si.on_wait and len(si.on_wait) > 1:
                waits = list(si.on_wait)
                for w in waits[:-1]:
                    d = mybir.InstDrain(
                        name=nc.get_next_instruction_name(),
                        ins=[],
                        outs=[],
                        bass_is_fusable=False,
                    )
                    d.engine = inst.engine
                    d.sync_info = mybir.SyncInfo(on_wait=[w], on_update=[])
                    new_insts.append(d)
                inst.sync_info = mybir.SyncInfo(
                    on_wait=[waits[-1]], on_update=list(si.on_update)
                )
            new_insts.append(inst)
        bb.instructions[:] = new_insts


def prep_core(img):
    """img: [3,512,512] f32 -> (xa, xb) each [128, FREE] fp16."""
    p = np.pad(img, ((0, 0), (FR, FR), (FR, FR)), mode="reflect")  # [3,518,518]
    pw = np.zeros((C, PH, PW), np.float16)
    pw[:, :, 1 : 1 + PH + 2 - 2] = 0  # noop, keep zeros
    pw[:, :, 1:519] = p.astype(np.float16)
    s0, s1, s2 = pw.strides
    av = np.lib.stride_tricks.as_strided(
        pw, shape=(128, C, HALO, PW), strides=(RPP * s1, s0, s1, s2)
    )
    xa = np.ascontiguousarray(av).reshape(128, FREE)
    bw = np.zeros_like(pw)
    bw[:, :, 0:519] = pw[:, :, 1:520]
    bv = np.lib.stride_tricks.as_strided(
        bw, shape=(128, C, HALO, PW), strides=(RPP * s1, s0, s1, s2)
    )
    xb = np.ascontiguousarray(bv).reshape(128, FREE)
    xab = np.concatenate([xa, xb], axis=1)  # [128, 2*FREE]

    # band workspace: top 9 + bottom 9 padded rows, col-partitioned
    # band[q, c, j, m] = pw[c, row(j), 4q+1+m]
    local = 4 * np.arange(128)[:, None] + 1 + np.arange(10)[None, :]  # [128,10]
    right = np.broadcast_to(np.arange(509, 519)[None, :], (128, 10))
    colidx = np.concatenate([local, right], axis=1)  # [128, 20]
    top = pw[:, 0:9, :][:, :, colidx]  # [3, 9, 128, 20]
    bot = pw[:, PH - 9 : PH, :][:, :, colidx]  # rows 509..517
    band = np.concatenate([top, bot], axis=1)  # [3, 18, 128, 10]
    band = np.ascontiguousarray(band.transpose(2, 0, 1, 3)).reshape(128, BAND_FREE)
    return xab, band


_CACHE = {}

# best measured config (173.2 us on HW, rel err 1.1e-4):
# symmetric pair formulation + corner taps dropped + channel sums on PE
BEST_KW = {"sym": True, "drop_r2": 18, "pe_sums": 2, "nbufs": 3}


def _get_runner(repeat=1, **build_kw):
    """Build the bass program once and return a cached jitted SPMD callable.

    Mirrors concourse.bass2jax.run_bass_via_pjrt, but keeps the jitted
    executable alive so repeated kernel() calls (and timing loops) reuse
    the compiled NEFF instead of re-tracing.
    """
    key = f"runner{repeat}|{sorted(build_kw.items())}"
    if key in _CACHE:
        return _CACHE[key]

    import jax
    from jax.experimental.shard_map import shard_map
    from jax.sharding import Mesh, PartitionSpec
    from concourse import bass2jax
    import concourse.mybir as mybir_

    bass2jax.install_neuronx_cc_hook()

    nc = build_nc(repeat=repeat, **build_kw)
    nc.finalize()

    in_names, out_names, out_avals, zero_outs = [], [], [], []
    partition_name = (
        nc.partition_id_tensor.name if nc.partition_id_tensor else None
    )
    for alloc in nc.m.functions[0].allocations:
        if not isinstance(alloc, mybir_.MemoryLocationSet):
            continue
        name = alloc.memorylocations[0].name
        if alloc.kind == "ExternalInput":
            if name != partition_name:
                in_names.append(name)
        elif alloc.kind == "ExternalOutput":
            out_names.append(name)
            shape = tuple(alloc.tensor_shape)
            dtype = mybir_.dt.np(alloc.dtype)
            out_avals.append(jax.core.ShapedArray(shape, dtype))
            zero_outs.append(np.zeros(shape, dtype))
    n_params = len(in_names)
    n_outs = len(out_avals)
    all_names = in_names + out_names
    if partition_name is not None:
        all_names.append(partition_name)
    donate = tuple(range(n_params, n_params + n_outs))

    def _body(*args):
        operands = list(args)
        if partition_name is not None:
            operands.append(bass2jax.partition_id_tensor())
        outs = bass2jax._bass_exec_p.bind(
            *operands,
            out_avals=tuple(out_avals),
            in_names=tuple(all_names),
            out_names=tuple(out_names),
            lowering_input_output_aliases=(),
            sim_require_finite=True,
            sim_require_nnan=True,
            nc=nc,
        )
        return tuple(outs)

    devices = jax.devices()[:B]
    mesh = Mesh(np.asarray(devices), ("core",))
    in_specs = (PartitionSpec("core"),) * (n_params + n_outs)
    out_specs = (PartitionSpec("core"),) * n_outs
    sharded = jax.jit(
        shard_map(
            _body, mesh=mesh, in_specs=in_specs, out_specs=out_specs,
            check_rep=False,
        ),
        donate_argnums=donate,
        keep_unused=True,
    )

    def run(in_maps, timing_reps=0):
        concat_in = [
            np.concatenate([np.asarray(m[name]) for m in in_maps], axis=0)
            for name in in_names
        ]
        concat_zeros = [
            np.zeros((B * z.shape[0], *z.shape[1:]), z.dtype) for z in zero_outs
        ]
        times = []
        if timing_reps > 0:
            # stage inputs on device once so repeat calls time dispatch+exec
            import time as _time

            sharding = jax.sharding.NamedSharding(mesh, PartitionSpec("core"))
            dev_in = [jax.device_put(a, sharding) for a in concat_in]
            for a in dev_in:
                a.block_until_ready()
            for _ in range(timing_reps):
                dz = [jax.device_put(z, sharding) for z in concat_zeros]
                for z in dz:
                    z.block_until_ready()
                t0 = _time.time()
                outs = sharded(*dev_in, *dz)
                for o in outs:
                    o.block_until_ready()
                times.append(_time.time() - t0)
        out_arrs = sharded(*concat_in, *concat_zeros)
        out_arrs = [np.asarray(o) for o in out_arrs]
        results = [
            {
                name: out_arrs[i].reshape(B, *out_avals[i].shape)[c]
                for i, name in enumerate(out_names)
            }
            for c in range(B)
        ]
        return results, times

    _CACHE[key] = run
    return run


def measure_exec_s(x, n=129, reps=8, **build_kw):
    """Time the kernel body on hardware via an on-device repeat loop.

    Builds two NEFFs: the normal one (repeat=1) and one whose offset sweep
    runs `n` times in a For_i loop.  (t_n - t_1) / (n - 1) cancels the
    dispatch/tunnel overhead, which dwarfs the kernel itself.  Calls are
    timed with device-staged inputs (timing_reps path).
    """
    in_maps = make_in_maps(x)
    results = {}
    for cnt in (1, n):
        run = _get_runner(repeat=cnt, **build_kw)
        _, times = run(in_maps, timing_reps=reps)
        results[cnt] = min(times)
    per_iter = (results[n] - results[1]) / (n - 1)
    return per_iter, results


def make_in_maps(x):
    maps = []
    ident = np.eye(128, dtype=np.float16)
    for b in range(B):
        xab, band = prep_core(x[b])
        maps.append({"xab": xab, "bands": band, "ident": ident})
    return maps


def run_on_cores(x, repeats=0):
    """x: [8,3,512,512] f32 numpy. Returns (loss, exec_times_s list)."""
    in_maps = make_in_maps(x)
    run = _get_runner(**BEST_KW)
    results, times = run(in_maps, timing_reps=repeats)
    total = 0.0
    for r in results:
        total += float(r["partial"].astype(np.float64).sum())
    loss = np.float32(total / (B * C * H * W))
    return loss, times


def kernel(input):
    x = np.asarray(input, dtype=np.float32)
    assert x.shape == (B, C, H, W), x.shape
    loss, _ = run_on_cores(x)
    return loss
